# revision 41
# baseline (speedup 1.0000x reference)
"""GQA causal self-attention (B=2, T=2048, C=2048, 16 Q heads / 4 KV heads,
head_dim=128) on 8 TRN2 NeuronCores.

Sharding: core = (batch b, kv-group g) for b in {0,1}, g in {0..3}.
Each core computes its batch's 4 Q heads that share KV head g, plus the
partial out-projection over those heads' rows of W_out. Host sums the 4
partials per batch (f32) and adds b_out.

Device layout (feature-major, "T" on the free axis):
  - QKV GEMM (q/k/v chunks) runs fp8e4m3 DoubleRow over c-chunk pairs.
  - qT/kT [d=128 part, t free] -> scores S^T[j,i] = kT_tile.T @ qT_slice
    in f32 PSUM pair tiles [128,2,512]; exp -> P bf16 in SBUF (single wide
    ACT op per pair); causal masking via memset + gpsimd affine_select.
  - P@V and the softmax denominator run bf16 (P direct, no fp8 cast): on
    TRN2 a bf16 matmul pair costs the same PE time as fp8 DoubleRow plus
    residual, and skipping the P->fp8 cast keeps ACT/DVE/GpSimd free.
  - attention processes heads in PAIRS (h, h+1 interleaved per j-pair) so
    PE always has independent work while ACT runs exp.
  - out-projection matmuls of slice s-1 are emitted as PE filler between
    attention pairs of slice s, hiding the exp->mask handoff latency.
  - output is DMA'd in bf16; host accumulates partials in f32.
"""

import sys

if "/opt/trn_rl_repo" not in sys.path:
    sys.path.insert(0, "/opt/trn_rl_repo")

import numpy as np
import ml_dtypes

BF16 = ml_dtypes.bfloat16
F8 = ml_dtypes.float8_e4m3

B = 2
T = 2048
C = 2048
NH = 16
NKV = 4
D = 128
GQ = NH // NKV  # 4 q heads per kv head
N_CORES = 8
CC = C // 128  # 16 contraction chunks
TS = T // 512  # 4 t-slices
TT = T // 128  # 16 t-tiles
NF = GQ + 2  # feature chunks per core: 4 q heads + k + v
NQK = GQ + 1  # q + k chunks (fp8 path); v stays bf16 for precision

_CACHED = {}


def _build_bass(reps=1, unroll=1):
    import concourse.bass as bass
    import concourse.bacc as bacc
    import concourse.tile as tile
    import concourse.mybir as mybir

    bf = mybir.dt.bfloat16
    f8 = mybir.dt.float8e4
    f32 = mybir.dt.float32
    Exp = mybir.ActivationFunctionType.Exp
    DR = mybir.MatmulPerfMode.DoubleRow

    nc = bacc.Bacc(None, target_bir_lowering=False)

    # DRAM inputs (host pre-laid-out, see kernel())
    xt8_d = nc.dram_tensor("xt8", [128, CC, T], f8, kind="ExternalInput")
    xtb_d = nc.dram_tensor("xtb", [128, CC, T], bf, kind="ExternalInput")
    wq8_d = nc.dram_tensor("wq8", [128, CC, NQK * 128], f8, kind="ExternalInput")
    wqv_d = nc.dram_tensor("wqv", [128, CC, 128], bf, kind="ExternalInput")
    bqkv_d = nc.dram_tensor("bqkv", [128, NF], f32, kind="ExternalInput")
    cos_d = nc.dram_tensor("cosT", [128, T], bf, kind="ExternalInput")
    sin_d = nc.dram_tensor("sinT", [128, T], bf, kind="ExternalInput")
    swap_d = nc.dram_tensor("swp", [128, 128], bf, kind="ExternalInput")
    iden_d = nc.dram_tensor("idn", [128, 128], bf, kind="ExternalInput")
    wout_d = nc.dram_tensor("wout", [128, GQ, C], bf, kind="ExternalInput")
    out_d = nc.dram_tensor("out", [T, C], bf, kind="ExternalOutput")

    with tile.TileContext(nc) as tc:
        with (
            tc.tile_pool(name="persist", bufs=1) as pers,
            tc.tile_pool(name="xt8", bufs=2) as x8p,
            tc.tile_pool(name="xtb", bufs=2) as xbp,
            tc.tile_pool(name="stage", bufs=3) as stg,
            tc.tile_pool(name="ptile", bufs=6) as ptp,
            tc.tile_pool(name="small", bufs=2) as smp,
            tc.tile_pool(name="osb", bufs=3) as osp,
            tc.tile_pool(name="ps_sc", bufs=2, space="PSUM") as pps,
            tc.tile_pool(name="ps_y", bufs=3, space="PSUM") as ppy,
            tc.tile_pool(name="ps_d", bufs=1, space="PSUM") as ppd,
        ):
            # ---- persistent loads (outside the timing loop) ----
            wq8_sb = pers.tile([128, CC, NQK * 128], f8)
            nc.sync.dma_start(wq8_sb[:], wq8_d[:])
            wqv_sb = pers.tile([128, CC, 128], bf)
            nc.sync.dma_start(wqv_sb[:], wqv_d[:])
            bq_sb = pers.tile([128, NF], f32)
            nc.sync.dma_start(bq_sb[:], bqkv_d[:])
            swap_sb = pers.tile([128, 128], bf)
            nc.sync.dma_start(swap_sb[:], swap_d[:])
            iden_sb = pers.tile([128, 128], bf)
            nc.sync.dma_start(iden_sb[:], iden_d[:])
            sin_sb = pers.tile([128, T], bf)
            nc.sync.dma_start(sin_sb[:], sin_d[:])
            cos_sb = pers.tile([128, T], bf)
            nc.sync.dma_start(cos_sb[:], cos_d[:])
            wout_sb = pers.tile([128, GQ, C], bf)
            nc.sync.dma_start(wout_sb[:], wout_d[:])
            ones_sb = pers.tile([128, 1], bf)
            nc.vector.memset(ones_sb[:], 1.0)

            # persistent activations
            qk_sb = pers.tile([128, GQ + 1, T], bf)  # rotated q0..q3, k
            v_sb = pers.tile([128, TT, 128], bf)  # v in [t-part, d] tiles
            y_sb = pers.tile([128, GQ, T], bf)  # y^T per head

            import contextlib
            loop_cm = tc.For_i(0, reps, 1) if reps > 1 else contextlib.nullcontext()
            with loop_cm:
                for _u in range(unroll):
                    _body(nc, tc, mybir, bf, f8, f32, Exp, DR,
                          x8p, xbp, stg, ptp, smp, osp, pps, ppy, ppd,
                          xt8_d, xtb_d, wq8_sb, wqv_sb, bq_sb, cos_sb, sin_sb,
                          swap_sb, iden_sb, wout_sb, ones_sb,
                          qk_sb, v_sb, y_sb, out_d)
    nc.compile()
    return nc


def _body(nc, tc, mybir, bf, f8, f32, Exp, DR,
          x8p, xbp, stg, ptp, smp, osp, pps, ppy, ppd,
          xt8_d, xtb_d, wq8_sb, wqv_sb, bq_sb, cos_sb, sin_sb,
          swap_sb, iden_sb, wout_sb, ones_sb,
          qk_sb, v_sb, y_sb, out_d):
    mul = mybir.AluOpType.mult
    add = mybir.AluOpType.add

    # ---- phase 1: QKV (q/k fp8 DoubleRow, v bf16) + RoPE + v transpose ----
    def p1_dma(ts):
        def go():
            tsl = slice(ts * 512, (ts + 1) * 512)
            xt8 = x8p.tile([128, CC, 512], f8, tag="xt8", name=f"xt8_{ts}")
            xtb = xbp.tile([128, CC, 512], bf, tag="xtb", name=f"xtb_{ts}")
            # split transfers so the first QKV matmuls start sooner
            nc.sync.dma_start(xt8[:, 0:8, :], xt8_d[:, 0:8, tsl])
            nc.sync.dma_start(xt8[:, 8:16, :], xt8_d[:, 8:16, tsl])
            nc.sync.dma_start(xtb[:, 0:8, :], xtb_d[:, 0:8, tsl])
            nc.sync.dma_start(xtb[:, 8:16, :], xtb_d[:, 8:16, tsl])
            p1_x[ts] = (xt8, xtb)
        return go

    def p1_main(ts, f):
        def go():
            xt8, xtb = p1_x[ts]
            ps = ppy.tile([128, 512], f32, tag="yps", name=f"ps{ts}_{f}")
            if f < NQK:
                for cp in range(CC // 2):
                    nc.tensor.matmul(
                        ps[:],
                        wq8_sb[:, 2 * cp : 2 * cp + 2, f * 128 : (f + 1) * 128],
                        xt8[:, 2 * cp : 2 * cp + 2, :],
                        start=(cp == 0),
                        stop=(cp == CC // 2 - 1),
                        perf_mode=DR,
                    )
            else:
                for cc in range(CC):
                    nc.tensor.matmul(
                        ps[:],
                        wqv_sb[:, cc, :],
                        xtb[:, cc, :],
                        start=(cc == 0),
                        stop=(cc == CC - 1),
                    )
            # bias add (also PSUM->SBUF move) on ACT, bf16 out
            raw = stg.tile([128, 512], bf, tag="raw", name=f"raw{ts}_{f}")
            nc.scalar.add(raw[:], ps[:], bq_sb[:, f : f + 1])
            p1_raw[(ts, f)] = raw
        return go

    def p1_rope(ts, f):
        def go():
            tsl = slice(ts * 512, (ts + 1) * 512)
            raw = p1_raw.pop((ts, f))
            if f < NF - 1:
                # rope: rot = raw*cos + swap(raw)*sinsign
                psw = pps.tile([128, 512], f32, tag="spr", name=f"psw{ts}_{f}")
                nc.tensor.matmul(psw[:], swap_sb[:], raw[:], start=True, stop=True)
                swb = stg.tile([128, 512], bf, tag="swb", name=f"swb{ts}_{f}")
                nc.scalar.copy(swb[:], psw[:])
                tmp = stg.tile([128, 512], bf, tag="ropetmp", name=f"tmp{ts}_{f}")
                nc.vector.tensor_tensor(tmp[:], swb[:], sin_sb[:, tsl], mul)
                nc.vector.tensor_tensor(
                    qk_sb[:, f, tsl], raw[:], cos_sb[:, tsl], mul
                )
                nc.vector.tensor_tensor(
                    qk_sb[:, f, tsl], qk_sb[:, f, tsl], tmp[:], add
                )
            else:
                # v: transpose [d, t] -> [t, d] via PE
                for k in range(4):
                    pst = pps.tile([128, 128], bf, tag="spr", name=f"pst{ts}_{k}")
                    nc.tensor.transpose(
                        pst[:], raw[:, k * 128 : (k + 1) * 128], iden_sb[:]
                    )
                    nc.any.tensor_copy(v_sb[:, ts * 4 + k, :], pst[:])
        return go

    p1_x = {}
    p1_raw = {}
    # ts=0 (and the ts=1 DMA prefetch) inline, software-pipelined so PE never
    # waits on the ACT bias-add; ts=1..3 become filler thunks drained during
    # s=0's attention.
    p1_dma(0)()
    p1_dma(1)()
    p1_main(0, 0)()
    for f in range(1, NF):
        p1_main(0, f)()
        p1_rope(0, f - 1)()
    p1_rope(0, NF - 1)()

    fill_q = []  # filler thunks: phase-1 (during s=0) / out-proj (s>=1)
    for ts in range(1, TS):
        if ts >= 2:
            fill_q.append(p1_dma(ts))
        fill_q.append(p1_main(ts, 0))
        for f in range(1, NF):
            fill_q.append(p1_main(ts, f))
            fill_q.append(p1_rope(ts, f - 1))
        fill_q.append(p1_rope(ts, NF - 1))

    def drain(n):
        for _ in range(min(n, len(fill_q))):
            fill_q.pop(0)()

    def make_outproj(s):
        thunks = []
        for tt in range(4 * s, 4 * s + 4):
            o_sb = osp.tile([128, C], bf, tag="osb", name=f"osb{tt}")

            def mk_es(tt, es, o_sb):
                def go():
                    pso = ppy.tile([128, 512], f32, tag="yps", name=f"pso{tt}_{es}")
                    for h in range(GQ):
                        nc.tensor.matmul(
                            pso[:],
                            y_sb[:, h, tt * 128 : (tt + 1) * 128],
                            wout_sb[:, h, es * 512 : (es + 1) * 512],
                            start=(h == 0),
                            stop=(h == GQ - 1),
                        )
                    if es % 2 == 0:
                        nc.vector.tensor_copy(
                            o_sb[:, es * 512 : (es + 1) * 512], pso[:]
                        )
                    else:
                        nc.scalar.copy(
                            o_sb[:, es * 512 : (es + 1) * 512], pso[:]
                        )
                return go

            for es in range(4):
                thunks.append(mk_es(tt, es, o_sb))

            def mk_dma(tt, o_sb):
                def go():
                    nc.sync.dma_start(out_d[tt * 128 : (tt + 1) * 128, :], o_sb[:])
                return go

            thunks.append(mk_dma(tt, o_sb))
        return thunks

    for s in range(TS):
        isl = slice(s * 512, (s + 1) * 512)
        npr = 2 * (s + 1)
        nsteps = npr * 2  # pair-steps in this slice (2 head-pairs)
        nfill = len(fill_q)
        step_i = 0
        for hp in range(2):
            heads = (2 * hp, 2 * hp + 1)
            psy = {hh: ppy.tile([128, 512], f32, tag="yps", name=f"psy{hh}")
                   for hh in heads}
            psdt = ppd.tile([33, 512], f32, tag="dps")  # head rows at 0 and 32
            for pr in range(npr):
                po = max(0, 256 * pr - 512 * s)
                offs = [max(0, 128 * (2 * pr + j01) - 512 * s) for j01 in range(2)]
                diag = 2 * pr >= 4 * s
                # scores: group by j-tile so the k-tile stationary is shared
                # by both heads' matmuls (fewer LDWEIGHTS switches)
                pss = {hh: pps.tile([128, 2, 512], f32, tag="spr",
                                    name=f"pss{hh}") for hh in heads}
                for j01 in range(2):
                    jt = 2 * pr + j01
                    for hh in heads:
                        nc.tensor.matmul(
                            pss[hh][:, j01, offs[j01]:512],
                            qk_sb[:, GQ, jt * 128 : (jt + 1) * 128],
                            qk_sb[:, hh, s * 512 + offs[j01] : (s + 1) * 512],
                            start=True,
                            stop=True,
                        )
                P = {}
                for hh in heads:
                    P[hh] = ptp.tile([128, 2, 512], bf, tag="P", name=f"P{hh}")
                    nc.scalar.activation(
                        P[hh][:, :, po:512], pss[hh][:, :, po:512], Exp,
                        scale=1.0 / 128.0
                    )
                    if diag:
                        # zero the odd tile's uncomputed gap, then triangular
                        # masks (keep where p <= col rel. to tile off)
                        if offs[1] > po:
                            nc.vector.memset(P[hh][:, 1, po : offs[1]], 0.0)
                        for j01 in range(2):
                            off = offs[j01]
                            nc.gpsimd.affine_select(
                                out=P[hh][:, j01, off : off + 128],
                                in_=P[hh][:, j01, off : off + 128],
                                pattern=[[1, 128]],
                                compare_op=mybir.AluOpType.is_ge,
                                fill=0.0,
                                base=0,
                                channel_multiplier=-1,
                            )
                # filler between the scores and P@V groups: PE chews these
                # while ACT runs the two exps (and masks land)
                step_i += 1
                target = nfill * step_i // nsteps
                drain(target - (nfill - len(fill_q)))
                # P@V grouped by j-tile (shared v stationary), then the
                # denominator matmuls (ones stationary loaded once for 4)
                for j01 in range(2):
                    jt = 2 * pr + j01
                    off = offs[j01]
                    for hh in heads:
                        nc.tensor.matmul(
                            psy[hh][:, off:512],
                            v_sb[:, jt, :],
                            P[hh][:, j01, off:512],
                            start=(pr == 0 and j01 == 0),
                            stop=(pr == npr - 1 and j01 == 1),
                        )
                for j01 in range(2):
                    off = offs[j01]
                    for hi, hh in enumerate(heads):
                        nc.tensor.matmul(
                            psdt[32 * hi : 32 * hi + 1, off:512],
                            ones_sb[:],
                            P[hh][:, j01, off:512],
                            start=(pr == 0 and j01 == 0),
                            stop=(pr == npr - 1 and j01 == 1),
                        )
            # head-pair epilogue: den reciprocal + normalize
            for hi, hh in enumerate(heads):
                rden = smp.tile([1, 512], f32, tag="rden", name=f"rden{hh}")
                nc.vector.reciprocal(rden[:], psdt[32 * hi : 32 * hi + 1, :])
                rdb = smp.tile([128, 512], f32, tag="rdb", name=f"rdb{hh}")
                nc.gpsimd.partition_broadcast(rdb[:], rden[:])
                nc.vector.tensor_tensor(y_sb[:, hh, isl], psy[hh][:], rdb[:], mul)
        drain(len(fill_q))  # any leftovers before y_sb of this slice is consumed
        fill_q = make_outproj(s)
    drain(len(fill_q))


def _host_prep(x, rope_cache, W_qkv, b_qkv, W_out):
    """Build the 8 per-core input dicts."""
    q_dim = NH * D  # 2048
    kv_dim = NKV * D  # 512

    # rope tables in [d, t] layout
    sin = rope_cache[:, 0::2].astype(np.float32)  # [T, 64]
    cos = rope_cache[:, 1::2].astype(np.float32)
    cos2T = np.empty((128, T), np.float32)
    sinsT = np.empty((128, T), np.float32)
    cos2T[0::2] = cos.T
    cos2T[1::2] = cos.T
    sinsT[0::2] = -sin.T
    sinsT[1::2] = sin.T

    swap = np.zeros((128, 128), BF16)
    idx = np.arange(128)
    swap[idx, idx ^ 1] = 1
    iden = np.eye(128, dtype=BF16)

    in_maps = []
    for b in range(B):
        xT = x[b].T.astype(np.float32)  # [C, T]
        xTr = xT.reshape(CC, 128, T).transpose(1, 0, 2)  # [128, CC, T]
        xt8 = np.ascontiguousarray(xTr.astype(F8))
        xtb = np.ascontiguousarray(xTr.astype(BF16))
        for g in range(NKV):
            cols = np.concatenate(
                [
                    np.arange(4 * g * D, (4 * g + 4) * D),  # 4 q heads
                    np.arange(q_dim + g * D, q_dim + (g + 1) * D),  # k head
                    np.arange(q_dim + kv_dim + g * D, q_dim + kv_dim + (g + 1) * D),
                ]
            )
            wq = W_qkv[:, cols].astype(np.float32)  # [C, 768]
            wqr = wq.reshape(CC, 128, NF * 128).transpose(1, 0, 2)  # [128, CC, 768]
            wq8 = np.ascontiguousarray(wqr[:, :, : NQK * 128].astype(F8))
            wqv = np.ascontiguousarray(wqr[:, :, NQK * 128 :].astype(BF16))
            bq = np.ascontiguousarray(
                b_qkv[cols].astype(np.float32).reshape(NF, 128).T
            )  # [128, NF]
            wo = W_out[4 * g * D : (4 * g + 4) * D, :].astype(BF16)  # [512, C]
            wo = np.ascontiguousarray(
                wo.reshape(GQ, 128, C).transpose(1, 0, 2)
            )  # [128, GQ, C]
            in_maps.append(
                {
                    "xt8": xt8,
                    "xtb": xtb,
                    "wq8": wq8,
                    "wqv": wqv,
                    "bqkv": bq,
                    "cosT": np.ascontiguousarray(cos2T.astype(BF16)),
                    "sinT": np.ascontiguousarray(sinsT.astype(BF16)),
                    "swp": swap,
                    "idn": iden,
                    "wout": wo,
                }
            )
    return in_maps


def kernel(x, rope_cache, W_qkv, b_qkv, W_out, b_out, _trace=False):
    from concourse.bass_utils import run_bass_kernel_spmd

    if "nc" not in _CACHED:
        _CACHED["nc"] = _build_bass()
    nc = _CACHED["nc"]

    in_maps = _host_prep(
        np.asarray(x), np.asarray(rope_cache), np.asarray(W_qkv),
        np.asarray(b_qkv), np.asarray(W_out),
    )
    res = run_bass_kernel_spmd(nc, in_maps, core_ids=list(range(N_CORES)), trace=_trace)
    _CACHED["last_result"] = res

    out = np.zeros((B, T, C), np.float32)
    for b in range(B):
        acc = res.results[b * NKV]["out"].astype(np.float32)
        for g in range(1, NKV):
            acc = acc + res.results[b * NKV + g]["out"].astype(np.float32)
        out[b] = acc + np.asarray(b_out)[None, :]
    return out
